# revision 13
# baseline (speedup 1.0000x reference)
"""Trainium2 Bass kernel for sparse-attention aspect pooling.

reference math (per batch row b):
    proj[a,l,h]  = sum_d x[l,d] * P[a,d,h]
    score[a,l]   = sum_{w,h} proj[a,l+w-1,h] * E[a,w,h]   (zero-padded window)
    attn[a,l]    = softmax_l(score)
    rep[a,h]     = sum_l attn[a,l] * proj[a,l,h]

Sharding: pure data parallel over batch (512 -> 64 per core x 8 cores).
Each core processes 2 batch rows per iteration, two iterations software-
pipelined as independent streams:
  - proj via col-tiled matmuls (K split 3x100): psum[0:64]=b0, [64:128]=b1,
    rows c=(h*5+a)
  - score via 3 shifted matmuls against a block-diagonal selector built from E;
    output rows replicated across h so pooling is a row-wise mult+reduce
  - softmax needs no max-subtraction (|score| < ~0.05 for this problem's scale)
Engine allocation: Sync=input DMA, ACT=exp + half the PSUM copy,
DVE=pooling + recip + half the copy, GpSimd=attn normalize + attn DMA out.
"""

import numpy as np
import ml_dtypes

import concourse.bass as bass
import concourse.mybir as mybir
import concourse.tile as tile
from concourse import bacc
from concourse.bass_utils import run_bass_kernel_spmd

BF16 = mybir.dt.bfloat16
F32 = mybir.dt.float32

N_CORES = 8
BATCH = 512
B_CORE = BATCH // N_CORES      # 64
L = 500
D = 300
KCH = 100                      # contraction chunk (3 x 100 = 300)
A = 5
H = 10
CTX = 3
C = A * H                      # 50 used rows, padded to 64 per batch row
PAIRS = B_CORE // 2            # 32 iterations, 2 batch rows each

_CACHE = {}


def _build():
    nc = bacc.Bacc(
        "TRN2", target_bir_lowering=False, debug=False, num_devices=N_CORES
    )
    xt_h = nc.dram_tensor("xt", [KCH, 3, B_CORE * L], BF16, kind="ExternalInput")
    wm_h = nc.dram_tensor("wm", [KCH, 3, 64], BF16, kind="ExternalInput")
    sel_h = nc.dram_tensor("sel", [128, CTX, 128], BF16, kind="ExternalInput")
    attn_h = nc.dram_tensor("attn_o", [PAIRS, 128, L], BF16, kind="ExternalOutput")
    rep_h = nc.dram_tensor("rep_o", [128, PAIRS], F32, kind="ExternalOutput")

    xt = xt_h.ap()
    wm = wm_h.ap()
    sel = sel_h.ap()
    attn_o = attn_h.ap()
    rep_o = rep_h.ap()

    from contextlib import ExitStack

    with tile.TileContext(nc) as tc, ExitStack() as ctx:
        singles = ctx.enter_context(tc.tile_pool(name="singles", bufs=1))
        xin = ctx.enter_context(tc.tile_pool(name="xin", bufs=2))
        psum = ctx.enter_context(tc.tile_pool(name="psum", bufs=1, space="PSUM"))
        work = ctx.enter_context(tc.tile_pool(name="work", bufs=2))
        small = ctx.enter_context(tc.tile_pool(name="small", bufs=4))

        wt = singles.tile([KCH, 3, 64], BF16)
        nc.sync.dma_start(out=wt[:, :, :], in_=wm[:, :, :])
        selt = singles.tile([128, CTX, 128], BF16)
        nc.sync.dma_start(out=selt[:, :, :], in_=sel[:, :, :])
        repS = singles.tile([128, PAIRS], F32)

        def dma_in(i, s):
            c0 = i * 2 * L
            x3 = xin.tile([KCH, 3, 2 * L], BF16, tag=f"x{s}")
            nc.sync.dma_start(out=x3[:, :, :], in_=xt[:, :, c0 : c0 + 2 * L])
            return x3

        def proj_mm(x3, s):
            pP = psum.tile([128, 512], F32, tag=f"pP{s}")
            for half in (0, 1):
                for k in range(3):
                    nc.tensor.matmul(
                        pP[64 * half : 64 * half + 64, 0:L],
                        wt[:, k, :],
                        x3[:, k, half * L : half * L + L],
                        start=(k == 0),
                        stop=(k == 2),
                    )
            return pP

        def copy_p(pP, s):
            # split the PSUM->SBUF bf16 cast across ACT and DVE
            sbP = work.tile([128, L], BF16, tag=f"sbP{s}")
            nc.scalar.copy(out=sbP[:, 0:250], in_=pP[:, 0:250])
            nc.vector.tensor_copy(out=sbP[:, 250:L], in_=pP[:, 250:L])
            return sbP

        def score_mm(sbP, s):
            pS = psum.tile([128, 512], F32, tag=f"pS{s}")
            nc.tensor.matmul(
                pS[:, 0:L], selt[:, 1, :], sbP[:, 0:L], start=True, stop=False
            )
            nc.tensor.matmul(
                pS[:, 1:L], selt[:, 0, :], sbP[:, 0 : L - 1], start=False, stop=False
            )
            nc.tensor.matmul(
                pS[:, 0 : L - 1], selt[:, 2, :], sbP[:, 1:L], start=False, stop=True
            )
            return pS

        def exp_act(pS, s):
            sbE = work.tile([128, L], BF16, tag=f"sbE{s}")
            den = small.tile([128, 1], F32, tag=f"den{s}")
            nc.scalar.activation(
                out=sbE[:, :],
                in_=pS[:, 0:L],
                func=mybir.ActivationFunctionType.Exp,
                accum_out=den[:, :],
            )
            return sbE, den

        def tail(i, sbP, sbE, den, s):
            invden = small.tile([128, 1], F32, tag=f"invden{s}")
            nc.vector.reciprocal(out=invden[:, :], in_=den[:, :])
            trash = work.tile([128, L], BF16, tag=f"trash{s}")
            nc.vector.scalar_tensor_tensor(
                out=trash[:, :],
                in0=sbE[:, :],
                scalar=invden[:, :],
                in1=sbP[:, :],
                op0=mybir.AluOpType.mult,
                op1=mybir.AluOpType.mult,
                accum_out=repS[:, i : i + 1],
            )
            attnS = work.tile([128, L], BF16, tag=f"attnS{s}")
            nc.vector.tensor_scalar_mul(attnS[:, :], sbE[:, :], invden[:, :])
            nc.gpsimd.dma_start(out=attn_o[i, :, :], in_=attnS[:, :])

        G = 4
        for g in range(PAIRS // G):
            ids = [G * g + j for j in range(G)]
            xs = [dma_in(i, j) for j, i in enumerate(ids)]
            pPs = [proj_mm(x3, j) for j, x3 in enumerate(xs)]
            sbPs = [copy_p(pP, j) for j, pP in enumerate(pPs)]
            pSs = [score_mm(sbP, j) for j, sbP in enumerate(sbPs)]
            eds = [exp_act(pS, j) for j, pS in enumerate(pSs)]
            for j, i in enumerate(ids):
                tail(i, sbPs[j], eds[j][0], eds[j][1], j)

        nc.sync.dma_start(out=rep_o[:, :], in_=repS[:, :])

    nc.compile()
    return nc


def _get_nc():
    if "nc" not in _CACHE:
        _CACHE["nc"] = _build()
    return _CACHE["nc"]


def _prep_inputs(review_emb, asp_embed, asp_proj):
    """Host-side shard + layout prep. Returns in_maps for run_bass_kernel_spmd."""
    x = np.asarray(review_emb, dtype=np.float32)
    E = np.asarray(asp_embed, dtype=np.float32).reshape(A, CTX, H)
    P = np.asarray(asp_proj, dtype=np.float32)

    # W[d, h*5+a] = P[a,d,h], padded to 64 cols, d split (100, 3)
    wm = np.zeros((D, 64), dtype=np.float32)
    wm[:, :C] = P.transpose(1, 2, 0).reshape(D, C)
    wm = np.ascontiguousarray(
        wm.reshape(3, KCH, 64).transpose(1, 0, 2)
    ).astype(ml_dtypes.bfloat16)

    # selector: S_w[(h'*5+a'), (h*5+a)] = delta(a'==a) * E[a,w,h']
    # block-diagonal duplicate for the two stacked batch rows
    sel = np.zeros((128, CTX, 128), dtype=np.float32)
    hh = np.arange(H)
    for w in range(CTX):
        S = np.zeros((64, 64), dtype=np.float32)
        for a in range(A):
            S[np.ix_(hh * A + a, hh * A + a)] = E[a, w, :][:, None]
        sel[0:64, w, 0:64] = S
        sel[64:128, w, 64:128] = S
    sel = sel.astype(ml_dtypes.bfloat16)

    in_maps = []
    for k in range(N_CORES):
        shard = x[k * B_CORE : (k + 1) * B_CORE]          # (64, 500, 300)
        xtk = shard.transpose(2, 0, 1).reshape(D, B_CORE * L)
        xtk = np.ascontiguousarray(
            xtk.reshape(3, KCH, B_CORE * L).transpose(1, 0, 2)
        ).astype(ml_dtypes.bfloat16)                      # (100, 3, 32000)
        in_maps.append({"xt": xtk, "wm": wm, "sel": sel})
    return in_maps


def _unshard(results):
    attn = np.empty((BATCH, A, L), dtype=np.float32)
    rep = np.empty((BATCH, A, H), dtype=np.float32)
    for k in range(N_CORES):
        buf = results[k]["attn_o"].astype(np.float32)      # [PAIRS, 128, L]
        ab = buf.reshape(PAIRS, 2, 64, L)[:, :, :A, :]     # [32, 2, 5, 500]
        attn[k * B_CORE : (k + 1) * B_CORE] = ab.reshape(B_CORE, A, L)
        r = results[k]["rep_o"].reshape(2, 64, PAIRS)[:, :C, :]  # [2, 50, 32]
        # r[p, h*5+a, i] -> rep[k*64 + 2*i + p, a, h]
        r = r.reshape(2, H, A, PAIRS).transpose(3, 0, 2, 1)      # [32, 2, 5, 10]
        rep[k * B_CORE : (k + 1) * B_CORE] = r.reshape(B_CORE, A, H)
    return attn, rep


def run_on_device(review_emb, asp_embed, asp_proj, trace=False, **kw):
    nc = _get_nc()
    in_maps = _prep_inputs(review_emb, asp_embed, asp_proj)
    res = run_bass_kernel_spmd(
        nc, in_maps, core_ids=list(range(N_CORES)), trace=trace, **kw
    )
    return res


def kernel(review_emb, asp_embed, asp_proj):
    res = run_on_device(review_emb, asp_embed, asp_proj, trace=False)
    return _unshard(res.results)


# revision 14
# speedup vs baseline: 1.0968x; 1.0968x over previous
"""Trainium2 Bass kernel for sparse-attention aspect pooling.

reference math (per batch row b):
    proj[a,l,h]  = sum_d x[l,d] * P[a,d,h]
    score[a,l]   = sum_{w,h} proj[a,l+w-1,h] * E[a,w,h]   (zero-padded window)
    attn[a,l]    = softmax_l(score)
    rep[a,h]     = sum_l attn[a,l] * proj[a,l,h]

Sharding: pure data parallel over batch (512 -> 64 per core x 8 cores).
Each core processes 2 batch rows per iteration, two iterations software-
pipelined as independent streams:
  - proj via col-tiled matmuls (K split 3x100): psum[0:64]=b0, [64:128]=b1,
    rows c=(h*5+a)
  - score via 3 shifted matmuls against a block-diagonal selector built from E;
    output rows replicated across h so pooling is a row-wise mult+reduce
  - softmax needs no max-subtraction (|score| < ~0.05 for this problem's scale)
Engine allocation: Sync=input DMA, ACT=exp + half the PSUM copy,
DVE=pooling + recip + half the copy, GpSimd=attn normalize + attn DMA out.
"""

import numpy as np
import ml_dtypes

import concourse.bass as bass
import concourse.mybir as mybir
import concourse.tile as tile
from concourse import bacc
from concourse.bass_utils import run_bass_kernel_spmd

BF16 = mybir.dt.bfloat16
F32 = mybir.dt.float32

N_CORES = 8
BATCH = 512
B_CORE = BATCH // N_CORES      # 64
L = 500
D = 300
KCH = 100                      # contraction chunk (3 x 100 = 300)
A = 5
H = 10
CTX = 3
C = A * H                      # 50 used rows, padded to 64 per batch row
PAIRS = B_CORE // 2            # 32 iterations, 2 batch rows each

_CACHE = {}


def _build():
    nc = bacc.Bacc(
        "TRN2", target_bir_lowering=False, debug=False, num_devices=N_CORES
    )
    xt_h = nc.dram_tensor("xt", [KCH, 3, B_CORE * L], BF16, kind="ExternalInput")
    wm_h = nc.dram_tensor("wm", [KCH, 3, 64], BF16, kind="ExternalInput")
    sel_h = nc.dram_tensor("sel", [128, CTX, 128], BF16, kind="ExternalInput")
    attn_h = nc.dram_tensor("attn_o", [PAIRS, 128, L], BF16, kind="ExternalOutput")
    rep_h = nc.dram_tensor("rep_o", [128, PAIRS], F32, kind="ExternalOutput")

    xt = xt_h.ap()
    wm = wm_h.ap()
    sel = sel_h.ap()
    attn_o = attn_h.ap()
    rep_o = rep_h.ap()

    from contextlib import ExitStack

    with tile.TileContext(nc) as tc, ExitStack() as ctx:
        singles = ctx.enter_context(tc.tile_pool(name="singles", bufs=1))
        xin = ctx.enter_context(tc.tile_pool(name="xin", bufs=2))
        psum = ctx.enter_context(tc.tile_pool(name="psum", bufs=1, space="PSUM"))
        work = ctx.enter_context(tc.tile_pool(name="work", bufs=2))
        small = ctx.enter_context(tc.tile_pool(name="small", bufs=4))

        wt = singles.tile([KCH, 3, 64], BF16)
        nc.sync.dma_start(out=wt[:, :, :], in_=wm[:, :, :])
        selt = singles.tile([128, CTX, 128], BF16)
        nc.sync.dma_start(out=selt[:, :, :], in_=sel[:, :, :])
        repS = singles.tile([128, PAIRS], F32)

        def dma_in(i, s, eng=None):
            c0 = i * 2 * L
            x3 = xin.tile([KCH, 3, 2 * L], BF16, tag=f"x{s}")
            (eng or nc.sync).dma_start(out=x3[:, :, :], in_=xt[:, :, c0 : c0 + 2 * L])
            return x3

        def proj_mm(x3, s):
            # k-major so the two column-group halves stream concurrently
            pP = psum.tile([128, 512], F32, tag=f"pP{s}")
            for k in range(3):
                for half in (0, 1):
                    nc.tensor.matmul(
                        pP[64 * half : 64 * half + 64, 0:L],
                        wt[:, k, :],
                        x3[:, k, half * L : half * L + L],
                        start=(k == 0),
                        stop=(k == 2),
                        skip_group_check=True,
                    )
            return pP

        def copy_p(pP, s):
            # split the PSUM->SBUF bf16 cast across ACT and DVE
            sbP = work.tile([128, L], BF16, tag=f"sbP{s}")
            nc.scalar.copy(out=sbP[:, 0:250], in_=pP[:, 0:250])
            nc.vector.tensor_copy(out=sbP[:, 250:L], in_=pP[:, 250:L])
            return sbP

        def score_mm(sbP, s):
            pS = psum.tile([128, 512], F32, tag=f"pS{s}")
            nc.tensor.matmul(
                pS[:, 0:L], selt[:, 1, :], sbP[:, 0:L], start=True, stop=False
            )
            nc.tensor.matmul(
                pS[:, 1:L], selt[:, 0, :], sbP[:, 0 : L - 1], start=False, stop=False
            )
            nc.tensor.matmul(
                pS[:, 0 : L - 1], selt[:, 2, :], sbP[:, 1:L], start=False, stop=True
            )
            return pS

        def exp_act(pS, s):
            sbE = work.tile([128, L], BF16, tag=f"sbE{s}")
            den = small.tile([128, 1], F32, tag=f"den{s}")
            nc.scalar.activation(
                out=sbE[:, :],
                in_=pS[:, 0:L],
                func=mybir.ActivationFunctionType.Exp,
                accum_out=den[:, :],
            )
            return sbE, den

        def tail(i, sbP, sbE, den, s):
            invden = small.tile([128, 1], F32, tag=f"invden{s}")
            nc.vector.reciprocal(out=invden[:, :], in_=den[:, :])
            trash = work.tile([128, L], BF16, tag=f"trash{s}")
            nc.vector.scalar_tensor_tensor(
                out=trash[:, :],
                in0=sbE[:, :],
                scalar=invden[:, :],
                in1=sbP[:, :],
                op0=mybir.AluOpType.mult,
                op1=mybir.AluOpType.mult,
                accum_out=repS[:, i : i + 1],
            )
            attnS = work.tile([128, L], BF16, tag=f"attnS{s}")
            nc.vector.tensor_scalar_mul(attnS[:, :], sbE[:, :], invden[:, :])
            nc.gpsimd.dma_start(out=attn_o[i, :, :], in_=attnS[:, :])

        G = 4
        ramp_engines = [nc.sync, nc.scalar, nc.gpsimd, nc.sync]
        for g in range(PAIRS // G):
            ids = [G * g + j for j in range(G)]
            engs = ramp_engines if g == 0 else [None] * G
            xs = [dma_in(i, j, engs[j]) for j, i in enumerate(ids)]
            pPs = [proj_mm(x3, j) for j, x3 in enumerate(xs)]
            sbPs = [copy_p(pP, j) for j, pP in enumerate(pPs)]
            pSs = [score_mm(sbP, j) for j, sbP in enumerate(sbPs)]
            eds = [exp_act(pS, j) for j, pS in enumerate(pSs)]
            for j, i in enumerate(ids):
                tail(i, sbPs[j], eds[j][0], eds[j][1], j)

        nc.sync.dma_start(out=rep_o[:, :], in_=repS[:, :])

    nc.compile()
    return nc


def _get_nc():
    if "nc" not in _CACHE:
        _CACHE["nc"] = _build()
    return _CACHE["nc"]


def _prep_inputs(review_emb, asp_embed, asp_proj):
    """Host-side shard + layout prep. Returns in_maps for run_bass_kernel_spmd."""
    x = np.asarray(review_emb, dtype=np.float32)
    E = np.asarray(asp_embed, dtype=np.float32).reshape(A, CTX, H)
    P = np.asarray(asp_proj, dtype=np.float32)

    # W[d, h*5+a] = P[a,d,h], padded to 64 cols, d split (100, 3)
    wm = np.zeros((D, 64), dtype=np.float32)
    wm[:, :C] = P.transpose(1, 2, 0).reshape(D, C)
    wm = np.ascontiguousarray(
        wm.reshape(3, KCH, 64).transpose(1, 0, 2)
    ).astype(ml_dtypes.bfloat16)

    # selector: S_w[(h'*5+a'), (h*5+a)] = delta(a'==a) * E[a,w,h']
    # block-diagonal duplicate for the two stacked batch rows
    sel = np.zeros((128, CTX, 128), dtype=np.float32)
    hh = np.arange(H)
    for w in range(CTX):
        S = np.zeros((64, 64), dtype=np.float32)
        for a in range(A):
            S[np.ix_(hh * A + a, hh * A + a)] = E[a, w, :][:, None]
        sel[0:64, w, 0:64] = S
        sel[64:128, w, 64:128] = S
    sel = sel.astype(ml_dtypes.bfloat16)

    in_maps = []
    for k in range(N_CORES):
        shard = x[k * B_CORE : (k + 1) * B_CORE]          # (64, 500, 300)
        xtk = shard.transpose(2, 0, 1).reshape(D, B_CORE * L)
        xtk = np.ascontiguousarray(
            xtk.reshape(3, KCH, B_CORE * L).transpose(1, 0, 2)
        ).astype(ml_dtypes.bfloat16)                      # (100, 3, 32000)
        in_maps.append({"xt": xtk, "wm": wm, "sel": sel})
    return in_maps


def _unshard(results):
    attn = np.empty((BATCH, A, L), dtype=np.float32)
    rep = np.empty((BATCH, A, H), dtype=np.float32)
    for k in range(N_CORES):
        buf = results[k]["attn_o"].astype(np.float32)      # [PAIRS, 128, L]
        ab = buf.reshape(PAIRS, 2, 64, L)[:, :, :A, :]     # [32, 2, 5, 500]
        attn[k * B_CORE : (k + 1) * B_CORE] = ab.reshape(B_CORE, A, L)
        r = results[k]["rep_o"].reshape(2, 64, PAIRS)[:, :C, :]  # [2, 50, 32]
        # r[p, h*5+a, i] -> rep[k*64 + 2*i + p, a, h]
        r = r.reshape(2, H, A, PAIRS).transpose(3, 0, 2, 1)      # [32, 2, 5, 10]
        rep[k * B_CORE : (k + 1) * B_CORE] = r.reshape(B_CORE, A, H)
    return attn, rep


def run_on_device(review_emb, asp_embed, asp_proj, trace=False, **kw):
    nc = _get_nc()
    in_maps = _prep_inputs(review_emb, asp_embed, asp_proj)
    res = run_bass_kernel_spmd(
        nc, in_maps, core_ids=list(range(N_CORES)), trace=trace, **kw
    )
    return res


def kernel(review_emb, asp_embed, asp_proj):
    res = run_on_device(review_emb, asp_embed, asp_proj, trace=False)
    return _unshard(res.results)


# revision 15
# speedup vs baseline: 1.1100x; 1.0121x over previous
"""Trainium2 Bass kernel for sparse-attention aspect pooling.

reference math (per batch row b):
    proj[a,l,h]  = sum_d x[l,d] * P[a,d,h]
    score[a,l]   = sum_{w,h} proj[a,l+w-1,h] * E[a,w,h]   (zero-padded window)
    attn[a,l]    = softmax_l(score)
    rep[a,h]     = sum_l attn[a,l] * proj[a,l,h]

Sharding: pure data parallel over batch (512 -> 64 per core x 8 cores).
Each core processes 2 batch rows per iteration, two iterations software-
pipelined as independent streams:
  - proj via col-tiled matmuls (K split 3x100): psum[0:64]=b0, [64:128]=b1,
    rows c=(h*5+a)
  - score via 3 shifted matmuls against a block-diagonal selector built from E;
    output rows replicated across h so pooling is a row-wise mult+reduce
  - softmax needs no max-subtraction (|score| < ~0.05 for this problem's scale)
Engine allocation: Sync=input DMA, ACT=exp + half the PSUM copy,
DVE=pooling + recip + half the copy, GpSimd=attn normalize + attn DMA out.
"""

import numpy as np
import ml_dtypes

import concourse.bass as bass
import concourse.mybir as mybir
import concourse.tile as tile
from concourse import bacc
from concourse.bass_utils import run_bass_kernel_spmd

BF16 = mybir.dt.bfloat16
F32 = mybir.dt.float32

N_CORES = 8
BATCH = 512
B_CORE = BATCH // N_CORES      # 64
L = 500
D = 300
KCH = 100                      # contraction chunk (3 x 100 = 300)
A = 5
H = 10
CTX = 3
C = A * H                      # 50 used rows, padded to 64 per batch row
PAIRS = B_CORE // 2            # 32 iterations, 2 batch rows each

_CACHE = {}


def _build():
    nc = bacc.Bacc(
        "TRN2", target_bir_lowering=False, debug=False, num_devices=N_CORES
    )
    xt_h = nc.dram_tensor("xt", [KCH, PAIRS, 3, 2 * L], BF16, kind="ExternalInput")
    wm_h = nc.dram_tensor("wm", [KCH, 3, 64], BF16, kind="ExternalInput")
    sel_h = nc.dram_tensor("sel", [128, CTX, 128], BF16, kind="ExternalInput")
    attn_h = nc.dram_tensor("attn_o", [PAIRS, 128, L], BF16, kind="ExternalOutput")
    rep_h = nc.dram_tensor("rep_o", [128, PAIRS], F32, kind="ExternalOutput")

    xt = xt_h.ap()
    wm = wm_h.ap()
    sel = sel_h.ap()
    attn_o = attn_h.ap()
    rep_o = rep_h.ap()

    from contextlib import ExitStack

    with tile.TileContext(nc) as tc, ExitStack() as ctx:
        singles = ctx.enter_context(tc.tile_pool(name="singles", bufs=1))
        xin = ctx.enter_context(tc.tile_pool(name="xin", bufs=2))
        psum = ctx.enter_context(tc.tile_pool(name="psum", bufs=1, space="PSUM"))
        work = ctx.enter_context(tc.tile_pool(name="work", bufs=2))
        small = ctx.enter_context(tc.tile_pool(name="small", bufs=4))

        wt = singles.tile([KCH, 3, 64], BF16)
        nc.sync.dma_start(out=wt[:, :, :], in_=wm[:, :, :])
        selt = singles.tile([128, CTX, 128], BF16)
        nc.sync.dma_start(out=selt[:, :, :], in_=sel[:, :, :])
        repS = singles.tile([128, PAIRS], F32)

        def dma_in(i, s, eng=None):
            x3 = xin.tile([KCH, 3, 2 * L], BF16, tag=f"x{s}")
            (eng or nc.sync).dma_start(out=x3[:, :, :], in_=xt[:, i, :, :])
            return x3

        def proj_mm(x3, s):
            # k-major so the two column-group halves stream concurrently
            pP = psum.tile([128, 512], F32, tag=f"pP{s}")
            for k in range(3):
                for half in (0, 1):
                    nc.tensor.matmul(
                        pP[64 * half : 64 * half + 64, 0:L],
                        wt[:, k, :],
                        x3[:, k, half * L : half * L + L],
                        start=(k == 0),
                        stop=(k == 2),
                        skip_group_check=True,
                    )
            return pP

        def copy_p(pP, s):
            # split the PSUM->SBUF bf16 cast across ACT and DVE
            sbP = work.tile([128, L], BF16, tag=f"sbP{s}")
            nc.scalar.copy(out=sbP[:, :], in_=pP[:, 0:L])
            return sbP

        def score_mm(sbP, s):
            pS = psum.tile([128, 512], F32, tag=f"pS{s}")
            nc.tensor.matmul(
                pS[:, 0:L], selt[:, 1, :], sbP[:, 0:L], start=True, stop=False
            )
            nc.tensor.matmul(
                pS[:, 1:L], selt[:, 0, :], sbP[:, 0 : L - 1], start=False, stop=False
            )
            nc.tensor.matmul(
                pS[:, 0 : L - 1], selt[:, 2, :], sbP[:, 1:L], start=False, stop=True
            )
            return pS

        def exp_act(pS, s):
            sbE = work.tile([128, L], BF16, tag=f"sbE{s}")
            den = small.tile([128, 1], F32, tag=f"den{s}")
            nc.scalar.activation(
                out=sbE[:, :],
                in_=pS[:, 0:L],
                func=mybir.ActivationFunctionType.Exp,
                accum_out=den[:, :],
            )
            return sbE, den

        def tail(i, sbP, sbE, den, s):
            invden = small.tile([128, 1], F32, tag=f"invden{s}")
            nc.vector.reciprocal(out=invden[:, :], in_=den[:, :])
            trash = work.tile([128, L], BF16, tag=f"trash{s}")
            nc.vector.scalar_tensor_tensor(
                out=trash[:, :],
                in0=sbE[:, :],
                scalar=invden[:, :],
                in1=sbP[:, :],
                op0=mybir.AluOpType.mult,
                op1=mybir.AluOpType.mult,
                accum_out=repS[:, i : i + 1],
            )
            attnS = work.tile([128, L], BF16, tag=f"attnS{s}")
            nc.vector.tensor_scalar_mul(attnS[:, :], sbE[:, :], invden[:, :])
            nc.gpsimd.dma_start(out=attn_o[i, :, :], in_=attnS[:, :])

        G = 4
        ramp_engines = [nc.sync, nc.scalar, nc.gpsimd, nc.sync]
        n_groups = PAIRS // G
        for g in range(n_groups):
            ids = [G * g + j for j in range(G)]
            engs = ramp_engines if g == 0 else [None] * G
            xs = [dma_in(i, j, engs[j]) for j, i in enumerate(ids)]
            pPs = [proj_mm(x3, j) for j, x3 in enumerate(xs)]
            sbPs = [copy_p(pP, j) for j, pP in enumerate(pPs)]
            if g < n_groups - 1:
                pSs = [score_mm(sbP, j) for j, sbP in enumerate(sbPs)]
                eds = [exp_act(pS, j) for j, pS in enumerate(pSs)]
                for j, i in enumerate(ids):
                    tail(i, sbPs[j], eds[j][0], eds[j][1], j)
            else:
                for j, i in enumerate(ids):
                    pS = score_mm(sbPs[j], j)
                    sbE, den = exp_act(pS, j)
                    tail(i, sbPs[j], sbE, den, j)

        nc.sync.dma_start(out=rep_o[:, :], in_=repS[:, :])

    nc.compile()
    return nc


def _get_nc():
    if "nc" not in _CACHE:
        _CACHE["nc"] = _build()
    return _CACHE["nc"]


def _prep_inputs(review_emb, asp_embed, asp_proj):
    """Host-side shard + layout prep. Returns in_maps for run_bass_kernel_spmd."""
    x = np.asarray(review_emb, dtype=np.float32)
    E = np.asarray(asp_embed, dtype=np.float32).reshape(A, CTX, H)
    P = np.asarray(asp_proj, dtype=np.float32)

    # W[d, h*5+a] = P[a,d,h], padded to 64 cols, d split (100, 3)
    wm = np.zeros((D, 64), dtype=np.float32)
    wm[:, :C] = P.transpose(1, 2, 0).reshape(D, C)
    wm = np.ascontiguousarray(
        wm.reshape(3, KCH, 64).transpose(1, 0, 2)
    ).astype(ml_dtypes.bfloat16)

    # selector: S_w[(h'*5+a'), (h*5+a)] = delta(a'==a) * E[a,w,h']
    # block-diagonal duplicate for the two stacked batch rows
    sel = np.zeros((128, CTX, 128), dtype=np.float32)
    hh = np.arange(H)
    for w in range(CTX):
        S = np.zeros((64, 64), dtype=np.float32)
        for a in range(A):
            S[np.ix_(hh * A + a, hh * A + a)] = E[a, w, :][:, None]
        sel[0:64, w, 0:64] = S
        sel[64:128, w, 64:128] = S
    sel = sel.astype(ml_dtypes.bfloat16)

    in_maps = []
    for k in range(N_CORES):
        shard = x[k * B_CORE : (k + 1) * B_CORE]          # (64, 500, 300)
        xtk = shard.transpose(2, 0, 1).reshape(D, B_CORE * L)
        # (100, PAIRS, 3, 1000): per-iteration fully contiguous per partition
        xtk = np.ascontiguousarray(
            xtk.reshape(3, KCH, PAIRS, 2 * L).transpose(1, 2, 0, 3)
        ).astype(ml_dtypes.bfloat16)
        in_maps.append({"xt": xtk, "wm": wm, "sel": sel})
    return in_maps


def _unshard(results):
    attn = np.empty((BATCH, A, L), dtype=np.float32)
    rep = np.empty((BATCH, A, H), dtype=np.float32)
    for k in range(N_CORES):
        buf = results[k]["attn_o"].astype(np.float32)      # [PAIRS, 128, L]
        ab = buf.reshape(PAIRS, 2, 64, L)[:, :, :A, :]     # [32, 2, 5, 500]
        attn[k * B_CORE : (k + 1) * B_CORE] = ab.reshape(B_CORE, A, L)
        r = results[k]["rep_o"].reshape(2, 64, PAIRS)[:, :C, :]  # [2, 50, 32]
        # r[p, h*5+a, i] -> rep[k*64 + 2*i + p, a, h]
        r = r.reshape(2, H, A, PAIRS).transpose(3, 0, 2, 1)      # [32, 2, 5, 10]
        rep[k * B_CORE : (k + 1) * B_CORE] = r.reshape(B_CORE, A, H)
    return attn, rep


def run_on_device(review_emb, asp_embed, asp_proj, trace=False, **kw):
    nc = _get_nc()
    in_maps = _prep_inputs(review_emb, asp_embed, asp_proj)
    res = run_bass_kernel_spmd(
        nc, in_maps, core_ids=list(range(N_CORES)), trace=trace, **kw
    )
    return res


def kernel(review_emb, asp_embed, asp_proj):
    res = run_on_device(review_emb, asp_embed, asp_proj, trace=False)
    return _unshard(res.results)
